# revision 11
# baseline (speedup 1.0000x reference)
"""Trainium2 Bass kernel: polar-BP left-message butterfly (nn_IterateLeftLayer).

Math per stage i (9..0), with L = left row i+1 (unclipped), R = right row i:
  out[pos] = w0 * ms(L[pos], L[neg] + R[neg])
  out[neg] = w1 * ms(L[pos], R[pos]) + L[neg]
where ms(x,y) = sign(x)sign(y)min(|x|,|y|), pos = {c: bit i of c == 0},
neg = pos + 2^i.  Final output = clip(left, +-10) with rows 0..9 replaced.

ms is computed exactly in sign-magnitude form:
  ms(x,y) = min_f32(x & MAG, y & MAG) | ((x ^ y) & SIGN)
Bitwise ops run in the DVE's exact integer path; the min runs on positive
floats (a pure selection, no rounding).  Note: int32 *arithmetic* ops (e.g.
int min) are NOT exact on the DVE -- operands convert through fp32 ALUs.

Sharding: pure data-parallel over batch, 512 rows per core on 8 cores.
Layout: batch on partitions (4 groups of 128 coalesced along the free axis
-> [128, 4096] tiles); the butterfly is pure strided access patterns.
"""

import sys

for _p in ("/opt/trn_rl_repo",):
    if _p not in sys.path:
        sys.path.insert(0, _p)

import numpy as np

import concourse.bass as bass
import concourse.tile as tile
from concourse import bacc, mybir
from concourse.bass_utils import run_bass_kernel_spmd

NUM_STAGES = 10
CODE = 1024
B = 4096
N_CORES = 8
P = 128
CLIP = 10.0
F32 = mybir.dt.float32
I32 = mybir.dt.int32
ALU = mybir.AluOpType
ACTF = mybir.ActivationFunctionType


def _halves(ap, i):
    """pos/neg strided views of a [P, W] row for stage i."""
    r = 1 << i
    v = ap.rearrange("p (m two r) -> p m two r", two=2, r=r)
    return v[:, :, 0, :], v[:, :, 1, :]


def build(nc, weights, bpc):
    """Emit the per-core kernel. weights: [(w0, w1)] * 10, bpc: batch rows/core."""
    g = bpc // P
    w = g * CODE
    h = w // 2

    right_d = nc.dram_tensor("right", [bpc, NUM_STAGES + 1, CODE], F32,
                             kind="ExternalInput")
    left10_d = nc.dram_tensor("left10", [bpc, CODE], F32, kind="ExternalInput")
    out_d = nc.dram_tensor("out", [bpc, NUM_STAGES, CODE], F32,
                           kind="ExternalOutput")

    with tile.TileContext(nc) as tc:
        with (
            tc.tile_pool(name="lo", bufs=2) as lo_pool,
            tc.tile_pool(name="rin", bufs=2) as r_pool,
            tc.tile_pool(name="tadd", bufs=1) as t_pool,
            tc.tile_pool(name="bm", bufs=1) as bm_pool,
            tc.tile_pool(name="am", bufs=1) as am_pool,
            tc.tile_pool(name="mm", bufs=1) as m_pool,
            tc.tile_pool(name="uu", bufs=1) as u_pool,
            tc.tile_pool(name="ms", bufs=1) as ms_pool,
            tc.tile_pool(name="oc", bufs=2) as oc_pool,
            tc.tile_pool(name="cst", bufs=1) as c_pool,
        ):
            sgn_t = c_pool.tile([P, 1], I32, tag="sgn")
            nc.vector.memset(sgn_t[:], -0x80000000)

            L = lo_pool.tile([P, w], F32, tag="lo")
            nc.sync.dma_start(
                L[:].rearrange("p (g c) -> p g c", g=g),
                left10_d.ap().rearrange("(g p) c -> p g c", p=P),
            )

            for i in reversed(range(NUM_STAGES)):
                w0, w1 = weights[i]
                R = r_pool.tile([P, w], F32)
                nc.sync.dma_start(
                    R[:].rearrange("p (g c) -> p g c", g=g),
                    right_d.ap()[:, i, :].rearrange("(g p) c -> p g c", p=P),
                )

                Lp, Ln = _halves(L[:], i)
                Rp, Rn = _halves(R[:], i)
                Lpi, _ = _halves(L[:].bitcast(I32), i)
                Rpi, _ = _halves(R[:].bitcast(I32), i)

                t = t_pool.tile([P, h], F32)
                nc.vector.tensor_add(t[:], Ln, Rn)
                ti = t[:].bitcast(I32)

                # magnitudes on ScalarE (offloads the DVE), bm = [|t| , |Rp|]
                bm = bm_pool.tile([P, w], F32)
                nc.scalar.activation(bm[:, :h], t[:], ACTF.Abs)
                nc.scalar.activation(bm[:, h:], Rp, ACTF.Abs)
                am = am_pool.tile([P, h], F32)   # |Lp| (shared A/B)
                nc.scalar.activation(am[:], Lp, ACTF.Abs)

                # min of magnitudes: fp32 min of positive floats (exact select)
                m = m_pool.tile([P, w], F32)
                nc.vector.tensor_tensor(m[:, :h], bm[:, :h], am[:], ALU.min)
                nc.vector.tensor_tensor(m[:, h:], bm[:, h:], am[:], ALU.min)

                # composite signs: u = [t ^ Lp , Rp ^ Lp]
                u = u_pool.tile([P, w], I32)
                nc.vector.tensor_tensor(u[:, :h], ti, Lpi, ALU.bitwise_xor)
                nc.vector.tensor_tensor(u[:, h:], Rpi, Lpi, ALU.bitwise_xor)

                # ms = (u & SIGN) | m   (one fused op over both halves)
                ms = ms_pool.tile([P, w], I32)
                nc.vector.scalar_tensor_tensor(
                    ms[:], u[:], sgn_t[:], m[:].bitcast(I32),
                    ALU.bitwise_and, ALU.bitwise_or)
                msA = ms[:, :h].bitcast(F32)
                msB = ms[:, h:].bitcast(F32)

                O = lo_pool.tile([P, w], F32, tag="lo")
                Op, On = _halves(O[:], i)
                # out[pos] = w0 * msA   (ScalarE: copy with scale)
                nc.scalar.activation(Op, msA, ACTF.Copy, bias=0.0,
                                     scale=float(w0))
                # out[neg] = w1 * msB + Ln
                nc.vector.scalar_tensor_tensor(
                    On, msB, float(w1), Ln, ALU.mult, ALU.add)

                oc = oc_pool.tile([P, w], F32)
                nc.vector.tensor_scalar(
                    oc[:], O[:], CLIP, -CLIP, ALU.min, ALU.max)
                nc.sync.dma_start(
                    out_d.ap()[:, i, :].rearrange("(g p) c -> p g c", p=P),
                    oc[:].rearrange("p (g c) -> p g c", g=g),
                )
                L = O


TRACE = False
LAST_RESULTS = None


def _make_nc(weights, bpc):
    nc = bacc.Bacc("TRN2", target_bir_lowering=False, debug=False)
    build(nc, weights, bpc)
    nc.compile()
    return nc


def kernel(right, left, left_weights, iter):
    right = np.asarray(right, dtype=np.float32)
    left = np.asarray(left, dtype=np.float32)
    wsel = np.asarray(left_weights, dtype=np.float32)[int(iter)]  # [10, 2]
    weights = [(float(wsel[i, 0]), float(wsel[i, 1])) for i in range(NUM_STAGES)]

    bpc = B // N_CORES
    nc = _make_nc(weights, bpc)

    in_maps = []
    for c in range(N_CORES):
        sl = slice(c * bpc, (c + 1) * bpc)
        in_maps.append({
            "right": np.ascontiguousarray(right[sl]),
            "left10": np.ascontiguousarray(left[sl, NUM_STAGES, :]),
        })
    global LAST_RESULTS
    LAST_RESULTS = run_bass_kernel_spmd(
        nc, in_maps, list(range(N_CORES)), trace=TRACE)
    res = LAST_RESULTS.results

    out = np.empty((B, NUM_STAGES + 1, CODE), np.float32)
    for c in range(N_CORES):
        out[c * bpc:(c + 1) * bpc, :NUM_STAGES, :] = res[c]["out"]
    out[:, NUM_STAGES, :] = np.clip(left[:, NUM_STAGES, :], -CLIP, CLIP)
    return out


# revision 12
# speedup vs baseline: 1.0736x; 1.0736x over previous
"""Trainium2 Bass kernel: polar-BP left-message butterfly (nn_IterateLeftLayer).

Math per stage i (9..0), with L = left row i+1 (unclipped), R = right row i:
  out[pos] = w0 * ms(L[pos], L[neg] + R[neg])
  out[neg] = w1 * ms(L[pos], R[pos]) + L[neg]
where ms(x,y) = sign(x)sign(y)min(|x|,|y|), pos = {c: bit i of c == 0},
neg = pos + 2^i.  Final output = clip(left, +-10) with rows 0..9 replaced.

ms is computed exactly in sign-magnitude form:
  ms(x,y) = min_f32(x & MAG, y & MAG) | ((x ^ y) & SIGN)
Bitwise ops run in the DVE's exact integer path; the min runs on positive
floats (a pure selection, no rounding).  Note: int32 *arithmetic* ops (e.g.
int min) are NOT exact on the DVE -- operands convert through fp32 ALUs.

Sharding: pure data-parallel over batch, 512 rows per core on 8 cores.
Layout: batch on partitions (4 groups of 128 coalesced along the free axis
-> [128, 4096] tiles); the butterfly is pure strided access patterns.
"""

import sys

for _p in ("/opt/trn_rl_repo",):
    if _p not in sys.path:
        sys.path.insert(0, _p)

import numpy as np

import concourse.bass as bass
import concourse.tile as tile
from concourse import bacc, mybir
from concourse.bass_utils import run_bass_kernel_spmd

NUM_STAGES = 10
CODE = 1024
B = 4096
N_CORES = 8
P = 128
CLIP = 10.0
F32 = mybir.dt.float32
I32 = mybir.dt.int32
ALU = mybir.AluOpType
ACTF = mybir.ActivationFunctionType


def _halves(ap, i):
    """pos/neg strided views of a [P, W] row for stage i."""
    r = 1 << i
    v = ap.rearrange("p (m two r) -> p m two r", two=2, r=r)
    return v[:, :, 0, :], v[:, :, 1, :]


def build(nc, weights, bpc):
    """Emit the per-core kernel. weights: [(w0, w1)] * 10, bpc: batch rows/core."""
    g = bpc // P
    w = g * CODE
    h = w // 2

    right_d = nc.dram_tensor("right", [bpc, NUM_STAGES + 1, CODE], F32,
                             kind="ExternalInput")
    left10_d = nc.dram_tensor("left10", [bpc, CODE], F32, kind="ExternalInput")
    out_d = nc.dram_tensor("out", [bpc, NUM_STAGES, CODE], F32,
                           kind="ExternalOutput")

    with tile.TileContext(nc) as tc:
        with (
            tc.tile_pool(name="lo", bufs=2) as lo_pool,
            tc.tile_pool(name="rin", bufs=2) as r_pool,
            tc.tile_pool(name="tadd", bufs=1) as t_pool,
            tc.tile_pool(name="bm", bufs=1) as bm_pool,
            tc.tile_pool(name="am", bufs=1) as am_pool,
            tc.tile_pool(name="mm", bufs=1) as m_pool,
            tc.tile_pool(name="uu", bufs=1) as u_pool,
            tc.tile_pool(name="ms", bufs=1) as ms_pool,
            tc.tile_pool(name="oc", bufs=2) as oc_pool,
            tc.tile_pool(name="cst", bufs=1) as c_pool,
        ):
            sgn_t = c_pool.tile([P, 1], I32, tag="sgn")
            nc.vector.memset(sgn_t[:], -0x80000000)

            L = lo_pool.tile([P, w], F32, tag="lo")
            nc.sync.dma_start(
                L[:].rearrange("p (g c) -> p g c", g=g),
                left10_d.ap().rearrange("(g p) c -> p g c", p=P),
            )

            for i in reversed(range(NUM_STAGES)):
                w0, w1 = weights[i]
                R = r_pool.tile([P, w], F32)
                nc.sync.dma_start(
                    R[:].rearrange("p (g c) -> p g c", g=g),
                    right_d.ap()[:, i, :].rearrange("(g p) c -> p g c", p=P),
                )

                Lp, Ln = _halves(L[:], i)
                Rp, Rn = _halves(R[:], i)
                Lpi, _ = _halves(L[:].bitcast(I32), i)
                Rpi, _ = _halves(R[:].bitcast(I32), i)

                t = t_pool.tile([P, h], F32)
                nc.vector.tensor_add(t[:], Ln, Rn)
                ti = t[:].bitcast(I32)

                # magnitudes on ScalarE (offloads the DVE), bm = [|t| , |Rp|]
                bm = bm_pool.tile([P, w], F32)
                nc.scalar.activation(bm[:, :h], t[:], ACTF.Abs)
                nc.scalar.activation(bm[:, h:], Rp, ACTF.Abs)
                am = am_pool.tile([P, h], F32)   # |Lp| (shared A/B)
                nc.scalar.activation(am[:], Lp, ACTF.Abs)

                # min of magnitudes: fp32 min of positive floats (exact select)
                m = m_pool.tile([P, w], F32)
                nc.vector.tensor_tensor(m[:, :h], bm[:, :h], am[:], ALU.min)
                nc.vector.tensor_tensor(m[:, h:], bm[:, h:], am[:], ALU.min)

                # composite signs: u = [t ^ Lp , Rp ^ Lp]
                u = u_pool.tile([P, w], I32)
                nc.vector.tensor_tensor(u[:, :h], ti, Lpi, ALU.bitwise_xor)
                nc.vector.tensor_tensor(u[:, h:], Rpi, Lpi, ALU.bitwise_xor)

                # ms = (u & SIGN) | m   (one fused op over both halves)
                ms = ms_pool.tile([P, w], I32)
                nc.vector.scalar_tensor_tensor(
                    ms[:], u[:], sgn_t[:], m[:].bitcast(I32),
                    ALU.bitwise_and, ALU.bitwise_or)
                msA = ms[:, :h].bitcast(F32)
                msB = ms[:, h:].bitcast(F32)

                O = lo_pool.tile([P, w], F32, tag="lo")
                Op, On = _halves(O[:], i)
                # out[pos] = w0 * msA   (ScalarE: copy with scale)
                nc.scalar.activation(Op, msA, ACTF.Copy, bias=0.0,
                                     scale=float(w0))
                # out[neg] = w1 * msB + Ln
                nc.vector.scalar_tensor_tensor(
                    On, msB, float(w1), Ln, ALU.mult, ALU.add)

                oc = oc_pool.tile([P, w], F32)
                nc.gpsimd.tensor_scalar(
                    oc[:], O[:], CLIP, -CLIP, ALU.min, ALU.max)
                nc.sync.dma_start(
                    out_d.ap()[:, i, :].rearrange("(g p) c -> p g c", p=P),
                    oc[:].rearrange("p (g c) -> p g c", g=g),
                )
                L = O


TRACE = False
LAST_RESULTS = None


def _make_nc(weights, bpc):
    nc = bacc.Bacc("TRN2", target_bir_lowering=False, debug=False)
    build(nc, weights, bpc)
    nc.compile()
    return nc


def kernel(right, left, left_weights, iter):
    right = np.asarray(right, dtype=np.float32)
    left = np.asarray(left, dtype=np.float32)
    wsel = np.asarray(left_weights, dtype=np.float32)[int(iter)]  # [10, 2]
    weights = [(float(wsel[i, 0]), float(wsel[i, 1])) for i in range(NUM_STAGES)]

    bpc = B // N_CORES
    nc = _make_nc(weights, bpc)

    in_maps = []
    for c in range(N_CORES):
        sl = slice(c * bpc, (c + 1) * bpc)
        in_maps.append({
            "right": np.ascontiguousarray(right[sl]),
            "left10": np.ascontiguousarray(left[sl, NUM_STAGES, :]),
        })
    global LAST_RESULTS
    LAST_RESULTS = run_bass_kernel_spmd(
        nc, in_maps, list(range(N_CORES)), trace=TRACE)
    res = LAST_RESULTS.results

    out = np.empty((B, NUM_STAGES + 1, CODE), np.float32)
    for c in range(N_CORES):
        out[c * bpc:(c + 1) * bpc, :NUM_STAGES, :] = res[c]["out"]
    out[:, NUM_STAGES, :] = np.clip(left[:, NUM_STAGES, :], -CLIP, CLIP)
    return out
